# revision 37
# baseline (speedup 1.0000x reference)
"""Trainium2 Bass kernel for nn_CubicSpline (embedding_lookup-style affine map).

Reference computes, for t in [0,1):
    w[n,i] = 1 - |t[n] - i|          (i = 0..62)
    out    = w @ cp[:63]             ([N,63] @ [63,128])

For t in [0,1] the triangular weights collapse algebraically:
    w[n,0] = 1 - t[n];   w[n,i] = t[n] + (1 - i)   (i >= 1)
so
    out[n,:] = t[n] * A + B
    A = sum_{i=1}^{62} cp[i] - cp[0]
    B = cp[0] + sum_{i=1}^{62} (1-i) * cp[i]

The device kernel therefore only needs to materialize a rank-1 affine map --
purely memory bound on the output write. The output leaves the device as
fp16 (halving HBM write traffic vs fp32; rel err ~3e-4 from the final
rounding, well inside tolerance) and is upcast to fp32 on the host.

Per-core layout (data-parallel over N across 8 cores, contiguous shards):
  * host packs the t-shard into 16 "phase" rows plus ones rows:
        t_aug[j, q] = t_shard[16*q + j]  (j<16)
  * each 2048-row output tile g is produced by one weight load
    (lhsT = t_aug[:, 128g:128g+128]) and four N=512 fp32 matmuls against
    constant block-diagonal rhs tiles holding A (per phase) and B (ones row),
    so PSUM directly holds t*A + B for 2048 consecutive output rows
    in [128 partitions x 2048] layout (partition q -> rows 16q..16q+15).
  * PSUM -> SBUF fp32->fp16 cast copy is split between VectorE and ScalarE.
  * each SBUF tile DMAs out as one fully contiguous 512 KB HBM write
    (128 partitions x 4 KB lines).
"""

import os
import sys
from contextlib import ExitStack

for _p in ("/opt/trn_rl_repo", "/root/.axon_site/_ro/trn_rl_repo"):
    if os.path.isdir(_p) and _p not in sys.path:
        sys.path.insert(0, _p)

import ml_dtypes
import numpy as np

import concourse.mybir as mybir
import concourse.tile as tile
from concourse import bacc
from concourse import bass_utils

N_TOTAL = 1_000_000
D = 128
NUM_CP = 64
N_CORES = 8

R = 16                   # output rows per partition per tile (= #phase rows)
# Contraction rows (all bf16; PSUM accumulates fp32):
#   rows 0..R-1    : t_hi phases   x A_hi diag
#   rows R..2R-1   : t_lo phases   x A_hi diag
#   rows 2R..3R-1  : t_hi phases   x A_lo diag
#   rows 3R, 3R+1  : ones          x B_hi, B_lo
# -> t*A + B to ~1e-6 rel (only t_lo*A_lo dropped). bf16 operands avoid the
# PE's fp32 HI/LO double-pass (2x matmul cost) and enable fast weight load.
K = 3 * R + 2
S = R // 4               # N=512 matmuls per psum tile (4 phases each)
TILE_ROWS = 128 * R      # rows per output tile
TILES = 62               # tiles per core (61 full + 1 overlapping the tail)
NPC = N_TOTAL // N_CORES          # rows per core (exact, no padding)
FULL_TILES = NPC // TILE_ROWS     # 61
TAIL_BASE = NPC - TILE_ROWS       # tile 61 overlaps tile 60 by 1976 rows
NEFF = TILES * TILE_ROWS          # rows fed through the pipeline per core
QTOT = NEFF // R                  # q-columns per core

F32 = mybir.dt.float32
F16 = mybir.dt.float16
BF16 = mybir.dt.bfloat16
NPBF16 = ml_dtypes.bfloat16


def dve_tiles_for(tiles):
    """Tile indices generated directly on the DVE (no PE / PSUM / ACT)."""
    return set(g for g in range(1, tiles, 4)) | ({11} if tiles > 11 else set())


def gp_tiles_for(tiles):
    """Tile indices generated on GPSIMD (SBUF-only, like the DVE tiles)."""
    return set(g for g in (10, 26, 38, 54) if g < tiles)


def direct_tiles_for(tiles):
    """All tiles generated without the PE, in order (shared t_dve layout)."""
    return sorted(dve_tiles_for(tiles) | gp_tiles_for(tiles))


def build_body(tc, out_ap, t_aug_ap, rhs_ap, t_dve_ap, tiles, qtot):
    """Tile-framework kernel body (shared by the real build and sim tests)."""
    nc = tc.nc
    # [g, 128, 2048] view of the output: tile g / partition q / free (w,d)
    # maps to row 2048g + 16q + w, col d -> fully contiguous 512KB per tile.
    # The last tile overlaps the previous one (same rows, same values) so the
    # per-core output is exactly NPC rows with no padding.
    nrows = out_ap.shape[0]
    full = min(tiles, nrows // TILE_ROWS)
    out_full = out_ap[: full * TILE_ROWS].rearrange(
        "(g q w) d -> g q (w d)", q=128, w=R
    )

    def out_t(g):
        if g < full:
            return out_full[g]
        assert g == full and tiles == full + 1
        return out_ap[nrows - TILE_ROWS :].rearrange("(q w) d -> q (w d)", w=R)

    dve_set = dve_tiles_for(tiles)
    gp_set = gp_tiles_for(tiles)
    direct = direct_tiles_for(tiles)
    n_dve = len(direct)

    with ExitStack() as ctx:
        cpool = ctx.enter_context(tc.tile_pool(name="cpool", bufs=1))
        opool = ctx.enter_context(tc.tile_pool(name="opool", bufs=16))
        gpool = ctx.enter_context(tc.tile_pool(name="gpool", bufs=2))
        g2pool = ctx.enter_context(tc.tile_pool(name="g2pool", bufs=2))
        # 4 x [128, 1024] fp32 = all 8 PSUM banks; half-tile granularity so a
        # slow copy stalls the PE by at most one half, not a whole tile.
        ppool = ctx.enter_context(tc.tile_pool(name="ppool", bufs=4, space="PSUM"))

        # The PE streams moving columns at a hard 1.2 GHz here (HAM never
        # ramps), capping PE output at 128 elem/cycle.  ~1/4 of the tiles are
        # therefore generated on the DVE instead (t*A then +B, fp32 ops with
        # an fp16-cast final write), while ACT (plus DVE for a few) casts the
        # PE tiles out of PSUM.  All DMA descriptor generation lives on the
        # otherwise-idle SP-HWDGE and gpsimd-SWDGE paths.
        #
        # Load order: the DVE-path constants (A/B replicas + t for the DVE
        # tiles, one merged tensor) land first as a single transfer on the
        # fast HWDGE ring, so the scarcest engine starts earliest; the PE's
        # rhs follows on the same ring, with t_aug in parallel on the other.
        dve_sb = cpool.tile([128, 2 * D + n_dve * R], F32)
        nc.sync.dma_start(dve_sb[:], t_dve_ap)
        ab_sb = dve_sb[:, : 2 * D]
        tdve_sb = dve_sb[:, 2 * D :]
        rhs_sb = cpool.tile([K, S * 512], BF16)
        nc.sync.dma_start(
            rhs_sb[:].rearrange("k (s n) -> k s n", s=S),
            rhs_ap.transpose([1, 0, 2]),
        )
        a_bc = ab_sb[:, :D].unsqueeze(1).broadcast_to([128, R, D])
        b_bc = ab_sb[:, D:].unsqueeze(1).broadcast_to([128, R, D])

        out_rings = [nc.sync, nc.gpsimd]

        # t_aug loads as independent tiles: a small first chunk (on its own
        # ring so it lands in parallel with the rhs load), then two big ones.
        ngroups = qtot // 128
        bounds = [0, 128]
        rest = ngroups - 1
        bounds.append(bounds[-1] + (rest // 2) * 128)
        bounds.append(ngroups * 128)
        chunk_rings = [nc.gpsimd, nc.gpsimd, nc.sync]
        t_tiles = []
        for c in range(len(bounds) - 1):
            lo, hi = bounds[c], bounds[c + 1]
            tt = cpool.tile([K, hi - lo], BF16, name=f"tch{c}", tag=f"tch{c}")
            chunk_rings[c].dma_start(tt[:], t_aug_ap[:, lo:hi])
            t_tiles.append(tt)

        def lhsT_for(g):
            col = g * 128
            for c in range(len(bounds) - 1):
                if col < bounds[c + 1]:
                    off = col - bounds[c]
                    return t_tiles[c][:, off : off + 128]
            raise AssertionError

        half = TILE_ROWS // 2
        direct_idx = {g: i for i, g in enumerate(direct)}
        for g in range(tiles):
            ob = opool.tile([128, TILE_ROWS], F16, name="ob")
            if g in dve_set or g in gp_set:
                i = direct_idx[g]
                t_bc = (
                    tdve_sb[:, R * i : R * (i + 1)]
                    .unsqueeze(2)
                    .broadcast_to([128, R, D])
                )
                if g in dve_set:
                    eng, pool, nm = nc.vector, gpool, "tmp"
                else:
                    eng, pool, nm = nc.gpsimd, g2pool, "tmp2"
                tmp = pool.tile([128, TILE_ROWS], F32, name=nm)
                tmp_v = tmp[:].rearrange("p (w d) -> p w d", w=R)
                eng.tensor_mul(tmp_v, t_bc, a_bc)
                ob_v = ob[:].rearrange("p (w d) -> p w d", w=R)
                eng.tensor_add(ob_v, tmp_v, b_bc)
            else:
                lhsT = lhsT_for(g)
                for h in range(2):
                    psum = ppool.tile([128, half], F32, name="psum")
                    for s in range(2):
                        nc.tensor.matmul(
                            psum[:, 512 * s : 512 * (s + 1)],
                            lhsT,
                            rhs_sb[:, half * h + 512 * s : half * h + 512 * (s + 1)],
                            start=True,
                            stop=True,
                        )
                    nc.scalar.copy(ob[:, half * h : half * (h + 1)], psum[:])
            # the last few transfers go on the fast HWDGE ring so the SWDGE
            # drain at pool close isn't gated on late Q7-issued DMAs.
            ring = nc.sync if g >= tiles - 6 else out_rings[g % 2]
            if g == full and tiles == full + 1:
                # tail tile: only the 72 rows not already written by tile 60
                # (rows TAIL_BASE+16q+w >= full*TILE_ROWS).
                cut = full * TILE_ROWS - (nrows - TILE_ROWS)  # 1976
                qc, wc = divmod(cut, R)  # 123, 8
                ring.dma_start(
                    out_ap[full * TILE_ROWS : nrows - (128 - qc - 1) * R].rearrange(
                        "(o w) d -> o (w d)", o=1
                    ),
                    ob[qc : qc + 1, wc * D :],
                )
                ring.dma_start(
                    out_ap[nrows - (128 - qc - 1) * R :].rearrange(
                        "(q w) d -> q (w d)", w=R
                    ),
                    ob[qc + 1 :, :],
                )
            else:
                ring.dma_start(out_t(g), ob[:])


def build_nc(tiles=TILES, nrows=NPC):
    qtot = tiles * TILE_ROWS // R
    n_dve = len(direct_tiles_for(tiles))
    nc = bacc.Bacc(
        "TRN2", target_bir_lowering=False, debug=False, num_devices=N_CORES
    )
    t_aug = nc.dram_tensor("t_aug", [K, qtot], BF16, kind="ExternalInput").ap()
    rhs_c = nc.dram_tensor("rhs_c", [S, K, 512], BF16, kind="ExternalInput").ap()
    t_dve = nc.dram_tensor(
        "t_dve", [128, 2 * D + n_dve * R], F32, kind="ExternalInput"
    ).ap()
    out = nc.dram_tensor("out", [nrows, D], F16, kind="ExternalOutput").ap()
    with tile.TileContext(nc) as tc:
        build_body(tc, out, t_aug, rhs_c, t_dve, tiles, qtot)
    nc.compile()
    return nc


def _split_bf16(x64):
    """hi/lo bf16 split of a float64 array: hi + lo ~= x to ~2^-17 rel."""
    hi = x64.astype(NPBF16)
    lo = (x64 - hi.astype(np.float64)).astype(NPBF16)
    return hi, lo


def affine_consts(control_points):
    """A, B ([128] float64) of the collapsed affine map out = t*A + B."""
    cp = np.asarray(control_points, dtype=np.float64)
    A = cp[1 : NUM_CP - 1].sum(axis=0) - cp[0]
    i = np.arange(1, NUM_CP - 1, dtype=np.float64)
    B = cp[0] + ((1.0 - i)[:, None] * cp[1 : NUM_CP - 1]).sum(axis=0)
    return A, B


def make_rhs(A, B):
    """Constant rhs tiles [S, K, 512] bf16 (see row layout at top)."""
    A_hi, A_lo = _split_bf16(A)
    B_hi, B_lo = _split_bf16(B)
    rhs = np.zeros((S, K, 512), NPBF16)
    for s in range(S):
        for m in range(4):
            j = m + 4 * s
            sl = slice(128 * m, 128 * (m + 1))
            rhs[s, j, sl] = A_hi
            rhs[s, R + j, sl] = A_hi
            rhs[s, 2 * R + j, sl] = A_lo
            rhs[s, 3 * R, sl] = B_hi
            rhs[s, 3 * R + 1, sl] = B_lo
    return rhs


def make_t_aug(t_shard):
    """[K, QTOT] bf16: t_hi, t_lo, t_hi phase rows + two ones rows."""
    qtot = t_shard.shape[0] // R
    t64 = t_shard.astype(np.float64)
    t_hi, t_lo = _split_bf16(t64)
    ph_hi = t_hi.reshape(qtot, R).T  # [8, qtot], ph[j, q] = t[8q+j]
    ph_lo = t_lo.reshape(qtot, R).T
    ones = np.ones((2, qtot), NPBF16)
    return np.ascontiguousarray(
        np.concatenate([ph_hi, ph_lo, ph_hi, ones], axis=0)
    )


_NC_CACHE = {}


def _get_nc():
    if "nc" not in _NC_CACHE:
        _NC_CACHE["nc"] = build_nc()
    return _NC_CACHE["nc"]


def make_t_eff(t_shard):
    """[NEFF] fp32: per-tile rows, with the tail tile overlapping tile 60."""
    return np.concatenate(
        [t_shard[: FULL_TILES * TILE_ROWS], t_shard[TAIL_BASE:]]
    )


def make_t_dve(t_eff, A, B):
    """[128, 2*D + n_dve*R] fp32: A/B replicated across partitions, then the
    DVE-generated tiles' t values partition-major."""
    ab = np.broadcast_to(
        np.concatenate([A, B]).astype(np.float32)[None, :], (128, 2 * D)
    )
    dve = direct_tiles_for(TILES)
    cols = [
        t_eff[TILE_ROWS * g : TILE_ROWS * (g + 1)].reshape(128, R) for g in dve
    ]
    return np.ascontiguousarray(
        np.concatenate([ab] + cols, axis=1), dtype=np.float32
    )


def prepare_in_maps(t, control_points):
    t = np.asarray(t, dtype=np.float32)
    A, B = affine_consts(control_points)
    rhs = make_rhs(A, B)
    t_clipped = np.clip(t, 0.0, 1.0)
    shards = t_clipped.reshape(N_CORES, NPC)
    maps = []
    for c in range(N_CORES):
        t_eff = make_t_eff(shards[c])
        maps.append(
            {
                "t_aug": make_t_aug(t_eff),
                "rhs_c": rhs,
                "t_dve": make_t_dve(t_eff, A, B),
            }
        )
    return maps


def kernel(t, control_points):
    t = np.asarray(t)
    assert t.shape == (N_TOTAL,), t.shape
    nc = _get_nc()
    in_maps = prepare_in_maps(t, control_points)
    res = bass_utils.run_bass_kernel_spmd(
        nc, in_maps, core_ids=list(range(N_CORES))
    )
    full = np.concatenate([res.results[c]["out"] for c in range(N_CORES)], axis=0)
    return full.astype(np.float32)


if __name__ == "__main__":
    t = np.random.default_rng(0).random(N_TOTAL, dtype=np.float32)
    cp = np.random.default_rng(1).normal(size=(NUM_CP, D)).astype(np.float32)
    out = kernel(t, cp)
    A, B = affine_consts(cp)
    expect = t.astype(np.float64)[:, None] * A[None, :] + B[None, :]
    err = np.abs(out - expect).max() / (np.abs(expect).max() + 1e-9)
    print("self-check max rel err:", err)



# revision 45
# speedup vs baseline: 1.1677x; 1.1677x over previous
"""Trainium2 Bass kernel for nn_CubicSpline (embedding_lookup-style affine map).

Reference computes, for t in [0,1):
    w[n,i] = 1 - |t[n] - i|          (i = 0..62)
    out    = w @ cp[:63]             ([N,63] @ [63,128])

For t in [0,1] the triangular weights collapse algebraically:
    w[n,0] = 1 - t[n];   w[n,i] = t[n] + (1 - i)   (i >= 1)
so
    out[n,:] = t[n] * A + B
    A = sum_{i=1}^{62} cp[i] - cp[0]
    B = cp[0] + sum_{i=1}^{62} (1-i) * cp[i]

The device kernel therefore only needs to materialize a rank-1 affine map --
purely memory bound on the output write. The output leaves the device as
fp16 (halving HBM write traffic vs fp32; rel err ~3e-4 from the final
rounding, well inside tolerance) and is upcast to fp32 on the host.

Per-core layout (data-parallel over N across 8 cores, contiguous shards):
  * host packs the t-shard into 16 "phase" rows plus ones rows:
        t_aug[j, q] = t_shard[16*q + j]  (j<16)
  * each 2048-row output tile g is produced by one weight load
    (lhsT = t_aug[:, 128g:128g+128]) and four N=512 fp32 matmuls against
    constant block-diagonal rhs tiles holding A (per phase) and B (ones row),
    so PSUM directly holds t*A + B for 2048 consecutive output rows
    in [128 partitions x 2048] layout (partition q -> rows 16q..16q+15).
  * PSUM -> SBUF fp32->fp16 cast copy is split between VectorE and ScalarE.
  * each SBUF tile DMAs out as one fully contiguous 512 KB HBM write
    (128 partitions x 4 KB lines).
"""

import os
import sys
from contextlib import ExitStack

for _p in ("/opt/trn_rl_repo", "/root/.axon_site/_ro/trn_rl_repo"):
    if os.path.isdir(_p) and _p not in sys.path:
        sys.path.insert(0, _p)

import ml_dtypes
import numpy as np

import concourse.mybir as mybir
import concourse.tile as tile
from concourse import bacc
from concourse import bass_utils

N_TOTAL = 1_000_000
D = 128
NUM_CP = 64
N_CORES = 8

R = 16                   # output rows per partition per tile (= #phase rows)
# Contraction rows (all bf16; PSUM accumulates fp32):
#   rows 0..R-1    : t_hi phases   x A_hi diag
#   rows R..2R-1   : t_lo phases   x A_hi diag
#   rows 2R..3R-1  : t_hi phases   x A_lo diag
#   rows 3R, 3R+1  : ones          x B_hi, B_lo
# -> t*A + B to ~1e-6 rel (only t_lo*A_lo dropped). bf16 operands avoid the
# PE's fp32 HI/LO double-pass (2x matmul cost) and enable fast weight load.
K = 3 * R + 2
S = R // 4               # N=512 matmuls per psum tile (4 phases each)
TILE_ROWS = 128 * R      # rows per output tile
TILES = 62               # tiles per core (61 full + 1 overlapping the tail)
NPC = N_TOTAL // N_CORES          # rows per core (exact, no padding)
FULL_TILES = NPC // TILE_ROWS     # 61
TAIL_BASE = NPC - TILE_ROWS       # tile 61 overlaps tile 60 by 1976 rows
NEFF = TILES * TILE_ROWS          # rows fed through the pipeline per core
QTOT = NEFF // R                  # q-columns per core

F32 = mybir.dt.float32
F16 = mybir.dt.float16
BF16 = mybir.dt.bfloat16
NPBF16 = ml_dtypes.bfloat16


def dve_tiles_for(tiles):
    """Tile indices generated directly on the DVE (no PE / PSUM / ACT)."""
    return set(g for g in range(1, tiles, 4))


def dve_copy_tiles_for(tiles):
    """PE tile indices whose PSUM->SBUF cast copy runs on DVE (not ACT)."""
    gen = dve_tiles_for(tiles)
    return set(g for g in range(3, tiles, 13) if g not in gen)


def gp_tiles_for(tiles):
    """GPSIMD generation is a net loss: its SBUF tensor ops hold the
    DVE<->GpSimd shared SBUF port lock and slow DVE tensor_tensor by ~40%
    (measured 2287 -> 3195 ns).  Keep empty."""
    return set()


def direct_tiles_for(tiles):
    """All tiles generated without the PE, in order (shared t_dve layout)."""
    return sorted(dve_tiles_for(tiles) | gp_tiles_for(tiles))


def pe_tiles_for(tiles):
    """Tiles that flow through the PE (t_aug holds phases only for these)."""
    gen = dve_tiles_for(tiles) | gp_tiles_for(tiles)
    return [g for g in range(tiles) if g not in gen]


def build_body(tc, out_ap, t_aug_ap, rhs_ap, t_dve_ap, tiles, qtot):
    """Tile-framework kernel body (shared by the real build and sim tests)."""
    nc = tc.nc
    # [g, 128, 2048] view of the output: tile g / partition q / free (w,d)
    # maps to row 2048g + 16q + w, col d -> fully contiguous 512KB per tile.
    # The last tile overlaps the previous one (same rows, same values) so the
    # per-core output is exactly NPC rows with no padding.
    nrows = out_ap.shape[0]
    full = min(tiles, nrows // TILE_ROWS)
    out_full = out_ap[: full * TILE_ROWS].rearrange(
        "(g q w) d -> g q (w d)", q=128, w=R
    )

    def out_t(g):
        if g < full:
            return out_full[g]
        assert g == full and tiles == full + 1
        return out_ap[nrows - TILE_ROWS :].rearrange("(q w) d -> q (w d)", w=R)

    dve_set = dve_tiles_for(tiles)
    dve_copy_set = dve_copy_tiles_for(tiles)
    direct = direct_tiles_for(tiles)
    n_dve = len(direct)

    with ExitStack() as ctx:
        cpool = ctx.enter_context(tc.tile_pool(name="cpool", bufs=1))
        opool = ctx.enter_context(tc.tile_pool(name="opool", bufs=16))
        gpool = ctx.enter_context(tc.tile_pool(name="gpool", bufs=2))
        # 4 x [128, 1024] fp32 = all 8 PSUM banks; half-tile granularity so a
        # slow copy stalls the PE by at most one half, not a whole tile.
        ppool = ctx.enter_context(tc.tile_pool(name="ppool", bufs=4, space="PSUM"))

        # The PE streams moving columns at a hard 1.2 GHz here (HAM never
        # ramps), capping PE output at 128 elem/cycle.  ~1/4 of the tiles are
        # therefore generated on the DVE instead (t*A then +B, fp32 ops with
        # an fp16-cast final write), while ACT (plus DVE for a few) casts the
        # PE tiles out of PSUM.  All DMA descriptor generation lives on the
        # otherwise-idle SP-HWDGE and gpsimd-SWDGE paths.
        #
        # Load order: the DVE-path constants (A/B replicas + t for the DVE
        # tiles, one merged tensor) land first as a single transfer on the
        # fast HWDGE ring, so the scarcest engine starts earliest; the PE's
        # rhs follows on the same ring, with t_aug in parallel on the other.
        dve_sb = cpool.tile([128, 2 * D + n_dve * R], F32)
        nc.sync.dma_start(dve_sb[:], t_dve_ap)
        ab_sb = dve_sb[:, : 2 * D]
        tdve_sb = dve_sb[:, 2 * D :]
        rhs_sb = cpool.tile([K, S * 512], BF16)
        nc.sync.dma_start(
            rhs_sb[:].rearrange("k (s n) -> k s n", s=S),
            rhs_ap.transpose([1, 0, 2]),
        )
        a_bc = ab_sb[:, :D].unsqueeze(1).broadcast_to([128, R, D])
        b_bc = ab_sb[:, D:].unsqueeze(1).broadcast_to([128, R, D])

        out_rings = [nc.sync, nc.gpsimd]

        # t_aug loads as independent tiles: a small first chunk (on its own
        # ring so it lands in parallel with the rhs load), then two big ones.
        ngroups = qtot // 128
        bounds = [0, 128]
        rest = ngroups - 1
        bounds.append(bounds[-1] + (rest // 2) * 128)
        bounds.append(ngroups * 128)
        chunk_rings = [nc.gpsimd, nc.gpsimd, nc.sync]
        t_tiles = []
        for c in range(len(bounds) - 1):
            lo, hi = bounds[c], bounds[c + 1]
            tt = cpool.tile([K, hi - lo], BF16, name=f"tch{c}", tag=f"tch{c}")
            chunk_rings[c].dma_start(tt[:], t_aug_ap[:, lo:hi])
            t_tiles.append(tt)

        pe_index = {g: j for j, g in enumerate(pe_tiles_for(tiles))}

        def lhsT_for(g):
            col = pe_index[g] * 128
            for c in range(len(bounds) - 1):
                if col < bounds[c + 1]:
                    off = col - bounds[c]
                    return t_tiles[c][:, off : off + 128]
            raise AssertionError

        half = TILE_ROWS // 2
        direct_idx = {g: i for i, g in enumerate(direct)}
        for g in range(tiles):
            ob = opool.tile([128, TILE_ROWS], F16, name="ob")
            if g in dve_set:
                i = direct_idx[g]
                t_bc = (
                    tdve_sb[:, R * i : R * (i + 1)]
                    .unsqueeze(2)
                    .broadcast_to([128, R, D])
                )
                tmp = gpool.tile([128, TILE_ROWS], F32, name="tmp")
                tmp_v = tmp[:].rearrange("p (w d) -> p w d", w=R)
                nc.vector.tensor_mul(tmp_v, t_bc, a_bc)
                ob_v = ob[:].rearrange("p (w d) -> p w d", w=R)
                nc.vector.tensor_add(ob_v, tmp_v, b_bc)
            else:
                lhsT = lhsT_for(g)
                copier = (
                    nc.vector.tensor_copy
                    if g in dve_copy_set
                    else nc.scalar.copy
                )
                for h in range(2):
                    psum = ppool.tile([128, half], F32, name="psum")
                    for s in range(2):
                        nc.tensor.matmul(
                            psum[:, 512 * s : 512 * (s + 1)],
                            lhsT,
                            rhs_sb[:, half * h + 512 * s : half * h + 512 * (s + 1)],
                            start=True,
                            stop=True,
                        )
                    copier(ob[:, half * h : half * (h + 1)], psum[:])
            # the last few transfers go on the fast HWDGE ring so the SWDGE
            # drain at pool close isn't gated on late Q7-issued DMAs.
            ring = nc.sync if g >= tiles - 6 else out_rings[g % 2]
            if g == full and tiles == full + 1:
                # tail tile: only the 72 rows not already written by tile 60
                # (rows TAIL_BASE+16q+w >= full*TILE_ROWS).
                cut = full * TILE_ROWS - (nrows - TILE_ROWS)  # 1976
                qc, wc = divmod(cut, R)  # 123, 8
                ring.dma_start(
                    out_ap[full * TILE_ROWS : nrows - (128 - qc - 1) * R].rearrange(
                        "(o w) d -> o (w d)", o=1
                    ),
                    ob[qc : qc + 1, wc * D :],
                )
                ring.dma_start(
                    out_ap[nrows - (128 - qc - 1) * R :].rearrange(
                        "(q w) d -> q (w d)", w=R
                    ),
                    ob[qc + 1 :, :],
                )
            else:
                ring.dma_start(out_t(g), ob[:])


def build_nc(tiles=TILES, nrows=NPC):
    qtot = len(pe_tiles_for(tiles)) * 128
    n_dve = len(direct_tiles_for(tiles))
    nc = bacc.Bacc(
        "TRN2", target_bir_lowering=False, debug=False, num_devices=N_CORES
    )
    t_aug = nc.dram_tensor("t_aug", [K, qtot], BF16, kind="ExternalInput").ap()
    rhs_c = nc.dram_tensor("rhs_c", [S, K, 512], BF16, kind="ExternalInput").ap()
    t_dve = nc.dram_tensor(
        "t_dve", [128, 2 * D + n_dve * R], F32, kind="ExternalInput"
    ).ap()
    out = nc.dram_tensor("out", [nrows, D], F16, kind="ExternalOutput").ap()
    with tile.TileContext(nc) as tc:
        build_body(tc, out, t_aug, rhs_c, t_dve, tiles, qtot)
    nc.compile()
    return nc


def _split_bf16(x64):
    """hi/lo bf16 split of a float64 array: hi + lo ~= x to ~2^-17 rel."""
    hi = x64.astype(NPBF16)
    lo = (x64 - hi.astype(np.float64)).astype(NPBF16)
    return hi, lo


def affine_consts(control_points):
    """A, B ([128] float64) of the collapsed affine map out = t*A + B."""
    cp = np.asarray(control_points, dtype=np.float64)
    A = cp[1 : NUM_CP - 1].sum(axis=0) - cp[0]
    i = np.arange(1, NUM_CP - 1, dtype=np.float64)
    B = cp[0] + ((1.0 - i)[:, None] * cp[1 : NUM_CP - 1]).sum(axis=0)
    return A, B


def make_rhs(A, B):
    """Constant rhs tiles [S, K, 512] bf16 (see row layout at top)."""
    A_hi, A_lo = _split_bf16(A)
    B_hi, B_lo = _split_bf16(B)
    rhs = np.zeros((S, K, 512), NPBF16)
    for s in range(S):
        for m in range(4):
            j = m + 4 * s
            sl = slice(128 * m, 128 * (m + 1))
            rhs[s, j, sl] = A_hi
            rhs[s, R + j, sl] = A_hi
            rhs[s, 2 * R + j, sl] = A_lo
            rhs[s, 3 * R, sl] = B_hi
            rhs[s, 3 * R + 1, sl] = B_lo
    return rhs


def make_t_aug(t_shard):
    """[K, QTOT] bf16: t_hi, t_lo, t_hi phase rows + two ones rows."""
    qtot = t_shard.shape[0] // R
    t64 = t_shard.astype(np.float64)
    t_hi, t_lo = _split_bf16(t64)
    ph_hi = t_hi.reshape(qtot, R).T  # [8, qtot], ph[j, q] = t[8q+j]
    ph_lo = t_lo.reshape(qtot, R).T
    ones = np.ones((2, qtot), NPBF16)
    return np.ascontiguousarray(
        np.concatenate([ph_hi, ph_lo, ph_hi, ones], axis=0)
    )


_NC_CACHE = {}


def _get_nc():
    if "nc" not in _NC_CACHE:
        _NC_CACHE["nc"] = build_nc()
    return _NC_CACHE["nc"]


def make_t_eff(t_shard):
    """[NEFF] fp32: per-tile rows, with the tail tile overlapping tile 60."""
    return np.concatenate(
        [t_shard[: FULL_TILES * TILE_ROWS], t_shard[TAIL_BASE:]]
    )


def make_t_dve(t_eff, A, B):
    """[128, 2*D + n_dve*R] fp32: A/B replicated across partitions, then the
    DVE-generated tiles' t values partition-major."""
    ab = np.broadcast_to(
        np.concatenate([A, B]).astype(np.float32)[None, :], (128, 2 * D)
    )
    dve = direct_tiles_for(TILES)
    cols = [
        t_eff[TILE_ROWS * g : TILE_ROWS * (g + 1)].reshape(128, R) for g in dve
    ]
    return np.ascontiguousarray(
        np.concatenate([ab] + cols, axis=1), dtype=np.float32
    )


def prepare_in_maps(t, control_points):
    t = np.asarray(t, dtype=np.float32)
    A, B = affine_consts(control_points)
    rhs = make_rhs(A, B)
    t_clipped = np.clip(t, 0.0, 1.0)
    shards = t_clipped.reshape(N_CORES, NPC)
    maps = []
    for c in range(N_CORES):
        t_eff = make_t_eff(shards[c])
        t_pe = np.concatenate(
            [
                t_eff[TILE_ROWS * g : TILE_ROWS * (g + 1)]
                for g in pe_tiles_for(TILES)
            ]
        )
        maps.append(
            {
                "t_aug": make_t_aug(t_pe),
                "rhs_c": rhs,
                "t_dve": make_t_dve(t_eff, A, B),
            }
        )
    return maps


def kernel(t, control_points):
    t = np.asarray(t)
    assert t.shape == (N_TOTAL,), t.shape
    nc = _get_nc()
    in_maps = prepare_in_maps(t, control_points)
    res = bass_utils.run_bass_kernel_spmd(
        nc, in_maps, core_ids=list(range(N_CORES))
    )
    full = np.concatenate([res.results[c]["out"] for c in range(N_CORES)], axis=0)
    return full.astype(np.float32)


if __name__ == "__main__":
    t = np.random.default_rng(0).random(N_TOTAL, dtype=np.float32)
    cp = np.random.default_rng(1).normal(size=(NUM_CP, D)).astype(np.float32)
    out = kernel(t, cp)
    A, B = affine_consts(cp)
    expect = t.astype(np.float64)[:, None] * A[None, :] + B[None, :]
    err = np.abs(out - expect).max() / (np.abs(expect).max() + 1e-9)
    print("self-check max rel err:", err)

